# revision 6
# baseline (speedup 1.0000x reference)
import sys

if '/opt/trn_rl_repo' not in sys.path:
    sys.path.insert(0, '/opt/trn_rl_repo')

import numpy as np
import ml_dtypes

import concourse.bacc as bacc
import concourse.mybir as mybir
from concourse.tile import TileContext
from concourse import bass_utils


def _install_ntff_shim():
    # Register the axon NTFF profile hook if the image's antenv lacks it.
    try:
        import antenv.axon_hooks  # noqa: F401
        return
    except ImportError:
        pass
    try:
        import types
        import trn_agent_boot.trn_boot as tb
        hook = tb._ntff_profile_via_ctypes('/opt/axon/libaxon_pjrt.so')
        if hook is None:
            return
        m = types.ModuleType('antenv.axon_hooks')
        m.get_axon_ntff_profile_hook = lambda: hook
        sys.modules['antenv.axon_hooks'] = m
        import antenv
        antenv.axon_hooks = m
        bass_utils.upload_artifacts = lambda d: "local://skipped"
    except Exception:
        pass

# ---------------- problem constants (hardcoded per spec) ----------------
N_NODES = 200000
D_IN = 64
D_OUT = 64
NUM_RELATIONS = 16
NUM_BASES = 8

N_CORES = 8
SHARD = 25000                 # nodes per core
N_BLK = 196                   # 128-node target blocks per shard (196*128=25088)
SHARD_PAD = N_BLK * 128
SLOT = 128                    # edges per segment slot (PSUM/S granularity)
GRP = 4096                    # xe / S slab columns per DMA
BANK_SLOTS = 8                # segment slots per PSUM bank ([128, 512] fp32)
import os as _os
ALT = _os.environ.get('K_ALT', '0') == '1'      # base-64 lhsT matmuls hang HW; keep off
R_ALL = NUM_RELATIONS + 1     # 17 (incl self row)

FP = mybir.dt.float32
BF = mybir.dt.bfloat16

S_DT = mybir.dt.float8e4
S_NP = ml_dtypes.float8_e4m3


# ---------------- host-side plan ----------------

def build_plan(source, target, edge_type):
    """Bucket directed edges by (target core, 128-node target block), group by
    relation within each block. Per-(block, rel) capacities are the exact max
    over cores (uniform SPMD program). Each (block, rel) segment gets one or
    more 128-edge PSUM 'slots' (msgs land at partitions 0..len-1 of their
    slot); the xe stream is the exact concatenation of segments, padded only at
    GRP slab boundaries so no segment straddles a slab."""
    src2 = np.concatenate([source, target]).astype(np.int64)
    tgt2 = np.concatenate([target, source]).astype(np.int64)
    et2 = np.concatenate([edge_type, edge_type]).astype(np.int64)

    core = tgt2 // SHARD
    loc = tgt2 % SHARD
    blk = loc // 128
    slot = loc % 128

    R = NUM_RELATIONS
    counts = np.zeros((N_CORES, N_BLK, R), np.int64)
    per_core = []
    for c in range(N_CORES):
        m = core == c
        b_c, r_c, s_c, src_c = blk[m], et2[m], slot[m], src2[m]
        key = b_c * R + r_c
        counts[c] = np.bincount(key, minlength=N_BLK * R).reshape(N_BLK, R)
        per_core.append((b_c, r_c, s_c, src_c, key))

    cap = counts.max(axis=0)                       # [N_BLK, R], exact

    # pass 1: slots per block (rel, len, gslot); S-matrix slot ids
    seg_slot0 = np.zeros((N_BLK, R), np.int64)
    raw_blocks = [[] for _ in range(N_BLK)]
    gslot = 0
    for b in range(N_BLK):
        for r in range(R):
            c = int(cap[b, r])
            if c == 0:
                continue
            seg_slot0[b, r] = gslot
            o = 0
            while o < c:
                ln = min(SLOT, c - o)
                raw_blocks[b].append((r, ln, gslot))
                gslot += 1
                o += ln
    n_slots = gslot

    # pass 2: xe column assignment. Slots pair up within a bank: pair (2i,2i+1)
    # shares a column range of width max(len); even slot -> partitions 0-63,
    # odd -> 64-127 (disjoint PE row groups let LDWEIGHTS overlap matmuls).
    # A bank's columns never straddle a GRP slab.
    blocks = [[] for _ in range(N_BLK)]
    cur = 0
    for b in range(N_BLK):
        sl = raw_blocks[b]
        for k0 in range(0, len(sl), BANK_SLOTS):
            chunk = sl[k0:k0 + BANK_SLOTS]
            if ALT:
                width = sum(max(chunk[i][1], chunk[i + 1][1] if i + 1 < len(chunk) else 0)
                            for i in range(0, len(chunk), 2))
            else:
                width = sum(s[1] for s in chunk)
            if cur // GRP != (cur + width - 1) // GRP:
                cur = (cur // GRP + 1) * GRP
            if ALT:
                for i in range(0, len(chunk), 2):
                    pw = chunk[i][1]
                    r, ln, g = chunk[i]
                    blocks[b].append((r, ln, 0, cur, g))
                    if i + 1 < len(chunk):
                        r2, ln2, g2 = chunk[i + 1]
                        blocks[b].append((r2, ln2, 1, cur, g2))
                        pw = max(pw, ln2)
                    cur += pw
            else:
                for (r, ln, g) in chunk:
                    blocks[b].append((r, ln, 0, cur, g))
                    cur += ln
    S_xe = -(-cur // GRP) * GRP

    # per-edge coordinates
    slot_half = np.zeros(n_slots, np.int64)
    slot_col = np.zeros(n_slots, np.int64)
    for b in range(N_BLK):
        for (r, ln, h, xc, g) in blocks[b]:
            slot_half[g] = h
            slot_col[g] = xc

    stream_src = []   # per core: (cols0, srcs0, cols1, srcs1)
    s_rows = []
    s_cols = []
    for c in range(N_CORES):
        b_c, r_c, s_c, src_c, key = per_core[c]
        order = np.argsort(key, kind='stable')
        ks = key[order]
        grp_first = np.concatenate([[0], np.flatnonzero(np.diff(ks)) + 1])
        starts_per_edge = np.repeat(
            grp_first, np.diff(np.concatenate([grp_first, [len(ks)]])))
        rank = np.arange(len(ks)) - starts_per_edge
        bb, rr = b_c[order], r_c[order]
        g = seg_slot0[bb, rr] + rank // SLOT
        row = rank % SLOT
        col = slot_col[g] + row
        h = slot_half[g]
        stream_src.append((col[h == 0], src_c[order][h == 0],
                           col[h == 1], src_c[order][h == 1]))
        s_rows.append(row.astype(np.int64))
        s_cols.append((g * 128 + s_c[order]).astype(np.int64))

    return blocks, n_slots, stream_src, s_rows, s_cols, S_xe


# ---------------- device program ----------------

def build_nc(blocks, n_slots, S_xe):
    build_nc._bi = 0
    nc = bacc.Bacc("TRN2", debug=False)
    G_xe = S_xe // GRP
    S_s = -(-(n_slots * 128) // GRP) * GRP
    G_s = S_s // GRP

    xe_d = nc.dram_tensor("xe", [G_xe, 64, GRP], BF, kind="ExternalInput")
    smat_d = nc.dram_tensor("smat", [G_s, 128, GRP], S_DT, kind="ExternalInput")
    xtown_d = nc.dram_tensor("xtown", [64, SHARD_PAD], BF, kind="ExternalInput")
    attT_d = nc.dram_tensor("attT", [NUM_BASES, R_ALL], FP, kind="ExternalInput")
    bas_d = nc.dram_tensor("bas", [NUM_BASES, D_IN * D_OUT], FP, kind="ExternalInput")
    w_stage_d = nc.dram_tensor("wstage", [R_ALL, D_IN * D_OUT], BF, kind="Internal")
    out_d = nc.dram_tensor("out", [128, N_BLK, D_OUT], BF, kind="ExternalOutput")

    with TileContext(nc) as tc:
        with (
            tc.tile_pool(name="const", bufs=1) as constp,
            tc.tile_pool(name="ssp", bufs=4) as ssp,
            tc.tile_pool(name="stp", bufs=4) as stp,
            tc.tile_pool(name="wps", bufs=1, space="PSUM") as wps,
            tc.tile_pool(name="mps", bufs=4, space="PSUM") as mps,
            tc.tile_pool(name="ups", bufs=3, space="PSUM") as ups,
        ):
            attT = constp.tile([NUM_BASES, R_ALL], FP)
            bas = constp.tile([NUM_BASES, D_IN * D_OUT], FP)
            w_all = constp.tile([128, R_ALL, D_OUT], BF)
            w_stage = constp.tile([R_ALL, D_IN * D_OUT], BF)
            xtown_sb = constp.tile([128, SHARD_PAD], BF)
            xe_ring = constp.tile([128, 3 * GRP], BF)
            out_sb = constp.tile([128, N_BLK, D_OUT], BF)

            nc.sync.dma_start(attT[:], attT_d[:])
            nc.sync.dma_start(bas[:], bas_d[:])
            nc.sync.dma_start(xtown_sb[0:64, :], xtown_d[:])
            # rows 64-127 of xe/xtown are zero so every main-loop matmul can
            # use a K=128 contraction: one PE tile config for msgs, self and
            # agg matmuls alike (config switches cost ~150ns each).
            nc.vector.memset(xtown_sb[64:128, :], 0.0)
            nc.vector.memset(xe_ring[64:128, :], 0.0)

            # zero the msgs PSUM buffers once: matmuls only write partitions
            # 0..len-1 of each slot, and the bank copy reads all 128 rows; the
            # leftover rows must be finite (S has zero rows there).
            for _ in range(4):
                z = mps.tile([128, BANK_SLOTS * D_OUT], FP, tag="bank")
                nc.vector.memset(z[:], 0.0)

            # prefetch the first xe/S slabs so the PE can start immediately
            xe_tiles = {}
            s_tiles = {}
            def get_xe(ge):
                if ge not in xe_tiles:
                    sl = ge % 3
                    nc.sync.dma_start(
                        xe_ring[0:64, sl * GRP:(sl + 1) * GRP], xe_d[ge, :, :])
                    xe_tiles[ge] = sl
                return xe_tiles[ge]
            def get_s(gs):
                if gs not in s_tiles:
                    t = ssp.tile([128, GRP], S_DT, tag="S", name=f"S{gs}")
                    nc.scalar.dma_start(t[:], smat_d[gs, :, :])
                    s_tiles[gs] = t
                return s_tiles[gs]
            get_xe(0)
            get_s(0)

            # ---- W = attT.T @ bases (staged through DRAM to transpose) ----
            for j in range(8):
                wp = wps.tile([R_ALL, 512], FP)
                nc.tensor.matmul(wp[:], attT[:], bas[:, j * 512:(j + 1) * 512],
                                 start=True, stop=True)
                nc.scalar.copy(w_stage[:, j * 512:(j + 1) * 512], wp[:])
            nc.sync.dma_start(w_stage_d[:], w_stage[:])
            w_re = w_stage_d.rearrange("r (d o) -> d r o", d=D_IN, o=D_OUT)
            nc.sync.dma_start(w_all[0:64, :, :], w_re)
            nc.sync.dma_start(w_all[64:128, :, :], w_re)

            # flat bank list for prefetch lookahead
            bank_list = []
            for _b in range(N_BLK):
                _sl = blocks[_b]
                for _k0 in range(0, len(_sl), BANK_SLOTS):
                    bank_list.append(_sl[_k0:_k0 + BANK_SLOTS])

            # ---- main loop over target blocks ----
            # Pipelined emission: a bank's S-aggregation matmuls are emitted
            # after the next bank's msgs matmuls so the PE keeps streaming
            # while the PSUM->SBUF msgs copy completes.
            flip = 0
            deferred = []
            for b in range(N_BLK):
                slots = blocks[b]
                ns = len(slots)
                U = ups.tile([128, D_OUT], FP, tag="U", name=f"U{b}")
                nc.tensor.matmul(
                    U[:], xtown_sb[:, b * 128:(b + 1) * 128],
                    w_all[:, NUM_RELATIONS, :],
                    start=True, stop=(ns == 0))

                for k0 in range(0, ns, BANK_SLOTS):
                    chunk = slots[k0:k0 + BANK_SLOTS]
                    kn = len(chunk)
                    bank = mps.tile([128, BANK_SLOTS * D_OUT], FP, tag="bank")
                    for j, (r, ln, h, xc, g) in enumerate(chunk):
                        ge, col = divmod(xc, GRP)
                        sl = get_xe(ge)
                        c0 = sl * GRP + col
                        nc.tensor.matmul(
                            bank[0:ln, j * D_OUT:(j + 1) * D_OUT],
                            xe_ring[:, c0:c0 + ln],
                            w_all[:, r, :],
                            start=True, stop=True)
                    msgs_sb = stp.tile([128, BANK_SLOTS * D_OUT], BF, tag="m")
                    # split the bank copy across both engines to halve its
                    # latency on the agg critical path
                    half = ((kn + 1) // 2) * D_OUT
                    wid = kn * D_OUT
                    if flip & 1:
                        nc.scalar.copy(msgs_sb[:, :half], bank[:, :half])
                        if wid > half:
                            nc.vector.tensor_copy(
                                msgs_sb[:, half:wid], bank[:, half:wid])
                    else:
                        nc.vector.tensor_copy(msgs_sb[:, :half], bank[:, :half])
                        if wid > half:
                            nc.scalar.copy(msgs_sb[:, half:wid], bank[:, half:wid])
                    flip += 1

                    # prefetch slabs needed two banks ahead
                    bank_i = getattr(build_nc, '_bi', 0)
                    if bank_i + 2 < len(bank_list):
                        for (_r, _ln, _h, _xc, _g) in bank_list[bank_i + 2]:
                            get_xe(_xc // GRP)
                            get_s((_g * 128) // GRP)
                    build_nc._bi = bank_i + 1

                    ss = []
                    for j, (r, ln, h, xc, g) in enumerate(chunk):
                        gs, scol = divmod(g * 128, GRP)
                        ss.append((get_s(gs), scol))

                    def agg(U=U, ss=ss, msgs_sb=msgs_sb, k0=k0, kn=kn, ns=ns):
                        for j in range(kn):
                            s_sb, scol = ss[j]
                            nc.tensor.matmul(
                                U[:], s_sb[:, scol:scol + 128],
                                msgs_sb[:, j * D_OUT:(j + 1) * D_OUT],
                                start=False, stop=(k0 + j == ns - 1))
                    deferred.append(agg)
                    while len(deferred) > 3:
                        deferred.pop(0)()

                def finish(U=U, b=b, flip=flip):
                    if flip & 1:
                        nc.scalar.copy(out_sb[:, b, :], U[:])
                    else:
                        nc.vector.tensor_copy(out_sb[:, b, :], U[:])
                deferred.append(finish)
                flip += 1

                # drop consumed slabs (keep the ones still in use)
                if b + 1 < N_BLK and blocks[b + 1]:
                    ge_next = blocks[b + 1][0][3] // GRP
                    gs_next = (blocks[b + 1][0][4] * 128) // GRP
                    for gk in list(xe_tiles):
                        if gk < ge_next:
                            del xe_tiles[gk]
                    for gk in list(s_tiles):
                        if gk < gs_next:
                            del s_tiles[gk]
            for fn in deferred:
                fn()

            nc.sync.dma_start(out_d[:], out_sb[:])

    nc.compile()
    return nc


# ---------------- top-level kernel ----------------

def kernel(x, node_keep_mask, source, target, edge_type, bases, att):
    x = np.asarray(x, np.float32)
    mask = np.asarray(node_keep_mask)
    bases = np.asarray(bases, np.float32)
    att = np.asarray(att, np.float32)

    blocks, n_slots, stream_src, s_rows, s_cols, S_xe = build_plan(
        np.asarray(source), np.asarray(target), np.asarray(edge_type))
    nc = build_nc(blocks, n_slots, S_xe)
    G_xe = S_xe // GRP
    S_s = -(-(n_slots * 128) // GRP) * GRP
    G_s = S_s // GRP

    x_bf = x.astype(ml_dtypes.bfloat16)
    xm = (x * mask[:, None].astype(np.float32)).astype(ml_dtypes.bfloat16)
    attT = np.ascontiguousarray(att.T)
    bas = np.ascontiguousarray(bases.reshape(NUM_BASES, -1))

    in_maps = []
    for c in range(N_CORES):
        cols0, srcs0, cols1, srcs1 = stream_src[c]
        assert len(cols1) == 0
        xe = np.zeros((64, S_xe), ml_dtypes.bfloat16)
        xe[:, cols0] = x_bf[srcs0].T
        xeg = np.ascontiguousarray(
            xe.reshape(64, G_xe, GRP).transpose(1, 0, 2))     # [G_xe, 64, GRP]
        smat = np.zeros((128, S_s), S_NP)
        smat[s_rows[c], s_cols[c]] = 1.0
        smat = np.ascontiguousarray(
            smat.reshape(128, G_s, GRP).transpose(1, 0, 2))   # [G_s, 128, GRP]
        xtown = np.zeros((64, SHARD_PAD), ml_dtypes.bfloat16)
        xtown[:, :SHARD] = xm[c * SHARD:(c + 1) * SHARD].T
        in_maps.append({
            "xe": xeg, "smat": smat, "xtown": xtown,
            "attT": attT, "bas": bas,
        })

    import os
    trace = os.environ.get("K_TRACE", "0") == "1"
    if trace:
        _install_ntff_shim()
    res = bass_utils.run_bass_kernel_spmd(
        nc, in_maps, core_ids=list(range(N_CORES)), trace=trace)
    kernel.last_res = res
    if trace and res.exec_time_ns is not None:
        print(f"HW exec time: {res.exec_time_ns} ns", flush=True)
        kernel.last_exec_time_ns = res.exec_time_ns

    out = np.zeros((N_NODES, D_OUT), np.float32)
    for c in range(N_CORES):
        o = np.asarray(res.results[c]["out"], np.float32)   # [128, N_BLK, 64]
        out[c * SHARD:(c + 1) * SHARD] = (
            o.transpose(1, 0, 2).reshape(SHARD_PAD, D_OUT)[:SHARD])
    return out



# revision 7
# speedup vs baseline: 1.0497x; 1.0497x over previous
import sys

if '/opt/trn_rl_repo' not in sys.path:
    sys.path.insert(0, '/opt/trn_rl_repo')

import numpy as np
import ml_dtypes

import concourse.bacc as bacc
import concourse.mybir as mybir
from concourse.tile import TileContext
from concourse import bass_utils


def _install_ntff_shim():
    # Register the axon NTFF profile hook if the image's antenv lacks it.
    try:
        import antenv.axon_hooks  # noqa: F401
        return
    except ImportError:
        pass
    try:
        import types
        import trn_agent_boot.trn_boot as tb
        hook = tb._ntff_profile_via_ctypes('/opt/axon/libaxon_pjrt.so')
        if hook is None:
            return
        m = types.ModuleType('antenv.axon_hooks')
        m.get_axon_ntff_profile_hook = lambda: hook
        sys.modules['antenv.axon_hooks'] = m
        import antenv
        antenv.axon_hooks = m
        bass_utils.upload_artifacts = lambda d: "local://skipped"
    except Exception:
        pass

# ---------------- problem constants (hardcoded per spec) ----------------
N_NODES = 200000
D_IN = 64
D_OUT = 64
NUM_RELATIONS = 16
NUM_BASES = 8

N_CORES = 8
SHARD = 25000                 # nodes per core
N_BLK = 196                   # 128-node target blocks per shard (196*128=25088)
SHARD_PAD = N_BLK * 128
SLOT = 128                    # edges per segment slot (PSUM/S granularity)
GRP = 4096                    # xe / S slab columns per DMA
BANK_SLOTS = 8                # segment slots per PSUM bank ([128, 512] fp32)
import os as _os
ALT = _os.environ.get('K_ALT', '0') == '1'      # base-64 lhsT matmuls hang HW; keep off
R_ALL = NUM_RELATIONS + 1     # 17 (incl self row)

FP = mybir.dt.float32
BF = mybir.dt.bfloat16

S_DT = mybir.dt.float8e4
S_NP = ml_dtypes.float8_e4m3


# ---------------- host-side plan ----------------

def build_plan(source, target, edge_type):
    """Bucket directed edges by (target core, 128-node target block), group by
    relation within each block. Per-(block, rel) capacities are the exact max
    over cores (uniform SPMD program). Each (block, rel) segment gets one or
    more 128-edge PSUM 'slots' (msgs land at partitions 0..len-1 of their
    slot); the xe stream is the exact concatenation of segments, padded only at
    GRP slab boundaries so no segment straddles a slab."""
    src2 = np.concatenate([source, target]).astype(np.int64)
    tgt2 = np.concatenate([target, source]).astype(np.int64)
    et2 = np.concatenate([edge_type, edge_type]).astype(np.int64)

    core = tgt2 // SHARD
    loc = tgt2 % SHARD
    blk = loc // 128
    slot = loc % 128

    R = NUM_RELATIONS
    counts = np.zeros((N_CORES, N_BLK, R), np.int64)
    per_core = []
    for c in range(N_CORES):
        m = core == c
        b_c, r_c, s_c, src_c = blk[m], et2[m], slot[m], src2[m]
        key = b_c * R + r_c
        counts[c] = np.bincount(key, minlength=N_BLK * R).reshape(N_BLK, R)
        per_core.append((b_c, r_c, s_c, src_c, key))

    cap = counts.max(axis=0)                       # [N_BLK, R], exact

    # pass 1: slots per block (rel, len, gslot); S-matrix slot ids
    seg_slot0 = np.zeros((N_BLK, R), np.int64)
    raw_blocks = [[] for _ in range(N_BLK)]
    gslot = 0
    for b in range(N_BLK):
        for r in range(R):
            c = int(cap[b, r])
            if c == 0:
                continue
            seg_slot0[b, r] = gslot
            o = 0
            while o < c:
                ln = min(SLOT, c - o)
                raw_blocks[b].append((r, ln, gslot))
                gslot += 1
                o += ln
    n_slots = gslot

    # pass 2: xe column assignment. Slots pair up within a bank: pair (2i,2i+1)
    # shares a column range of width max(len); even slot -> partitions 0-63,
    # odd -> 64-127 (disjoint PE row groups let LDWEIGHTS overlap matmuls).
    # A bank's columns never straddle a GRP slab.
    blocks = [[] for _ in range(N_BLK)]
    cur = 0
    for b in range(N_BLK):
        sl = raw_blocks[b]
        for k0 in range(0, len(sl), BANK_SLOTS):
            chunk = sl[k0:k0 + BANK_SLOTS]
            if ALT:
                width = sum(max(chunk[i][1], chunk[i + 1][1] if i + 1 < len(chunk) else 0)
                            for i in range(0, len(chunk), 2))
            else:
                width = sum(s[1] for s in chunk)
            if cur // GRP != (cur + width - 1) // GRP:
                cur = (cur // GRP + 1) * GRP
            if ALT:
                for i in range(0, len(chunk), 2):
                    pw = chunk[i][1]
                    r, ln, g = chunk[i]
                    blocks[b].append((r, ln, 0, cur, g))
                    if i + 1 < len(chunk):
                        r2, ln2, g2 = chunk[i + 1]
                        blocks[b].append((r2, ln2, 1, cur, g2))
                        pw = max(pw, ln2)
                    cur += pw
            else:
                for (r, ln, g) in chunk:
                    blocks[b].append((r, ln, 0, cur, g))
                    cur += ln
    S_xe = -(-cur // GRP) * GRP

    # per-edge coordinates
    slot_half = np.zeros(n_slots, np.int64)
    slot_col = np.zeros(n_slots, np.int64)
    for b in range(N_BLK):
        for (r, ln, h, xc, g) in blocks[b]:
            slot_half[g] = h
            slot_col[g] = xc

    stream_src = []   # per core: (cols0, srcs0, cols1, srcs1)
    s_rows = []
    s_cols = []
    for c in range(N_CORES):
        b_c, r_c, s_c, src_c, key = per_core[c]
        order = np.argsort(key, kind='stable')
        ks = key[order]
        grp_first = np.concatenate([[0], np.flatnonzero(np.diff(ks)) + 1])
        starts_per_edge = np.repeat(
            grp_first, np.diff(np.concatenate([grp_first, [len(ks)]])))
        rank = np.arange(len(ks)) - starts_per_edge
        bb, rr = b_c[order], r_c[order]
        g = seg_slot0[bb, rr] + rank // SLOT
        row = rank % SLOT
        col = slot_col[g] + row
        h = slot_half[g]
        stream_src.append((col[h == 0], src_c[order][h == 0],
                           col[h == 1], src_c[order][h == 1]))
        s_rows.append(row.astype(np.int64))
        s_cols.append((g * 128 + s_c[order]).astype(np.int64))

    return blocks, n_slots, stream_src, s_rows, s_cols, S_xe


# ---------------- device program ----------------

def build_nc(blocks, n_slots, S_xe):
    build_nc._bi = 0
    nc = bacc.Bacc("TRN2", debug=False)
    G_xe = S_xe // GRP
    S_s = -(-(n_slots * 128) // GRP) * GRP
    G_s = S_s // GRP

    xe_d = nc.dram_tensor("xe", [G_xe, 64, GRP], BF, kind="ExternalInput")
    smat_d = nc.dram_tensor("smat", [G_s, 128, GRP], S_DT, kind="ExternalInput")
    xtown_d = nc.dram_tensor("xtown", [64, SHARD_PAD], BF, kind="ExternalInput")
    attT_d = nc.dram_tensor("attT", [NUM_BASES, R_ALL], FP, kind="ExternalInput")
    bas_d = nc.dram_tensor("bas", [NUM_BASES, D_IN * D_OUT], FP, kind="ExternalInput")
    w_stage_d = nc.dram_tensor("wstage", [R_ALL, D_IN * D_OUT], BF, kind="Internal")
    out_d = nc.dram_tensor("out", [128, N_BLK, D_OUT], BF, kind="ExternalOutput")

    with TileContext(nc) as tc:
        with (
            tc.tile_pool(name="const", bufs=1) as constp,
            tc.tile_pool(name="ssp", bufs=4) as ssp,
            tc.tile_pool(name="stp", bufs=4) as stp,
            tc.tile_pool(name="wps", bufs=1, space="PSUM") as wps,
            tc.tile_pool(name="mps", bufs=4, space="PSUM") as mps,
            tc.tile_pool(name="ups", bufs=3, space="PSUM") as ups,
        ):
            attT = constp.tile([NUM_BASES, R_ALL], FP)
            bas = constp.tile([NUM_BASES, D_IN * D_OUT], FP)
            w_all = constp.tile([128, R_ALL, D_OUT], BF)
            w_stage = constp.tile([R_ALL, D_IN * D_OUT], BF)
            xtown_sb = constp.tile([128, SHARD_PAD], BF)
            xe_ring = constp.tile([128, 3 * GRP], BF)
            out_sb = constp.tile([128, N_BLK, D_OUT], BF)

            nc.sync.dma_start(attT[:], attT_d[:])
            nc.sync.dma_start(bas[:], bas_d[:])
            nc.sync.dma_start(xtown_sb[0:64, :], xtown_d[:])
            # rows 64-127 of xe/xtown are zero so every main-loop matmul can
            # use a K=128 contraction: one PE tile config for msgs, self and
            # agg matmuls alike (config switches cost ~150ns each).
            nc.gpsimd.memset(xtown_sb[64:128, :], 0.0)
            nc.gpsimd.memset(xe_ring[64:128, :], 0.0)

            # zero the msgs PSUM buffers once: matmuls only write partitions
            # 0..len-1 of each slot, and the bank copy reads all 128 rows; the
            # leftover rows must be finite (S has zero rows there).
            for _ in range(4):
                z = mps.tile([128, BANK_SLOTS * D_OUT], FP, tag="bank")
                nc.vector.memset(z[:], 0.0)

            # prefetch the first xe/S slabs so the PE can start immediately
            xe_tiles = {}
            s_tiles = {}
            def get_xe(ge):
                if ge not in xe_tiles:
                    sl = ge % 3
                    nc.sync.dma_start(
                        xe_ring[0:64, sl * GRP:(sl + 1) * GRP], xe_d[ge, :, :])
                    xe_tiles[ge] = sl
                return xe_tiles[ge]
            def get_s(gs):
                if gs not in s_tiles:
                    t = ssp.tile([128, GRP], S_DT, tag="S", name=f"S{gs}")
                    nc.gpsimd.dma_start(t[:], smat_d[gs, :, :])
                    s_tiles[gs] = t
                return s_tiles[gs]
            get_xe(0)
            get_s(0)

            # ---- W = attT.T @ bases (staged through DRAM to transpose) ----
            for j in range(8):
                wp = wps.tile([R_ALL, 512], FP)
                nc.tensor.matmul(wp[:], attT[:], bas[:, j * 512:(j + 1) * 512],
                                 start=True, stop=True)
                nc.scalar.copy(w_stage[:, j * 512:(j + 1) * 512], wp[:])
            nc.sync.dma_start(w_stage_d[:], w_stage[:])
            w_re = w_stage_d.rearrange("r (d o) -> d r o", d=D_IN, o=D_OUT)
            nc.sync.dma_start(w_all[0:64, :, :], w_re)
            nc.sync.dma_start(w_all[64:128, :, :], w_re)

            # flat bank list for prefetch lookahead
            bank_list = []
            for _b in range(N_BLK):
                _sl = blocks[_b]
                for _k0 in range(0, len(_sl), BANK_SLOTS):
                    bank_list.append(_sl[_k0:_k0 + BANK_SLOTS])

            # ---- main loop over target blocks ----
            # Pipelined emission: a bank's S-aggregation matmuls are emitted
            # after the next bank's msgs matmuls so the PE keeps streaming
            # while the PSUM->SBUF msgs copy completes.
            flip = 0
            deferred = []
            for b in range(N_BLK):
                slots = blocks[b]
                ns = len(slots)
                U = ups.tile([128, D_OUT], FP, tag="U", name=f"U{b}")
                nc.tensor.matmul(
                    U[:], xtown_sb[:, b * 128:(b + 1) * 128],
                    w_all[:, NUM_RELATIONS, :],
                    start=True, stop=(ns == 0))

                for k0 in range(0, ns, BANK_SLOTS):
                    chunk = slots[k0:k0 + BANK_SLOTS]
                    kn = len(chunk)
                    bank = mps.tile([128, BANK_SLOTS * D_OUT], FP, tag="bank")
                    for j, (r, ln, h, xc, g) in enumerate(chunk):
                        ge, col = divmod(xc, GRP)
                        sl = get_xe(ge)
                        c0 = sl * GRP + col
                        nc.tensor.matmul(
                            bank[0:ln, j * D_OUT:(j + 1) * D_OUT],
                            xe_ring[:, c0:c0 + ln],
                            w_all[:, r, :],
                            start=True, stop=True)
                    msgs_sb = stp.tile([128, BANK_SLOTS * D_OUT], BF, tag="m")
                    # split the bank copy across both engines to halve its
                    # latency on the agg critical path
                    half = ((kn + 1) // 2) * D_OUT
                    wid = kn * D_OUT
                    if flip & 1:
                        nc.scalar.copy(msgs_sb[:, :half], bank[:, :half])
                        if wid > half:
                            nc.vector.tensor_copy(
                                msgs_sb[:, half:wid], bank[:, half:wid])
                    else:
                        nc.vector.tensor_copy(msgs_sb[:, :half], bank[:, :half])
                        if wid > half:
                            nc.scalar.copy(msgs_sb[:, half:wid], bank[:, half:wid])
                    flip += 1

                    # prefetch slabs needed two banks ahead
                    bank_i = getattr(build_nc, '_bi', 0)
                    if bank_i + 2 < len(bank_list):
                        for (_r, _ln, _h, _xc, _g) in bank_list[bank_i + 2]:
                            get_xe(_xc // GRP)
                            get_s((_g * 128) // GRP)
                    build_nc._bi = bank_i + 1

                    ss = []
                    for j, (r, ln, h, xc, g) in enumerate(chunk):
                        gs, scol = divmod(g * 128, GRP)
                        ss.append((get_s(gs), scol))

                    def agg(U=U, ss=ss, msgs_sb=msgs_sb, k0=k0, kn=kn, ns=ns):
                        for j in range(kn):
                            s_sb, scol = ss[j]
                            nc.tensor.matmul(
                                U[:], s_sb[:, scol:scol + 128],
                                msgs_sb[:, j * D_OUT:(j + 1) * D_OUT],
                                start=False, stop=(k0 + j == ns - 1))
                    deferred.append(agg)
                    while len(deferred) > 4:
                        deferred.pop(0)()

                def finish(U=U, b=b, flip=flip):
                    if flip & 1:
                        nc.scalar.copy(out_sb[:, b, :], U[:])
                    else:
                        nc.vector.tensor_copy(out_sb[:, b, :], U[:])
                deferred.append(finish)
                flip += 1

                # drop consumed slabs (keep the ones still in use)
                if b + 1 < N_BLK and blocks[b + 1]:
                    ge_next = blocks[b + 1][0][3] // GRP
                    gs_next = (blocks[b + 1][0][4] * 128) // GRP
                    for gk in list(xe_tiles):
                        if gk < ge_next:
                            del xe_tiles[gk]
                    for gk in list(s_tiles):
                        if gk < gs_next:
                            del s_tiles[gk]
            for fn in deferred:
                fn()

            nc.sync.dma_start(out_d[:], out_sb[:])

    nc.compile()
    return nc


# ---------------- top-level kernel ----------------

def kernel(x, node_keep_mask, source, target, edge_type, bases, att):
    x = np.asarray(x, np.float32)
    mask = np.asarray(node_keep_mask)
    bases = np.asarray(bases, np.float32)
    att = np.asarray(att, np.float32)

    blocks, n_slots, stream_src, s_rows, s_cols, S_xe = build_plan(
        np.asarray(source), np.asarray(target), np.asarray(edge_type))
    nc = build_nc(blocks, n_slots, S_xe)
    G_xe = S_xe // GRP
    S_s = -(-(n_slots * 128) // GRP) * GRP
    G_s = S_s // GRP

    x_bf = x.astype(ml_dtypes.bfloat16)
    xm = (x * mask[:, None].astype(np.float32)).astype(ml_dtypes.bfloat16)
    attT = np.ascontiguousarray(att.T)
    bas = np.ascontiguousarray(bases.reshape(NUM_BASES, -1))

    in_maps = []
    for c in range(N_CORES):
        cols0, srcs0, cols1, srcs1 = stream_src[c]
        assert len(cols1) == 0
        xe = np.zeros((64, S_xe), ml_dtypes.bfloat16)
        xe[:, cols0] = x_bf[srcs0].T
        xeg = np.ascontiguousarray(
            xe.reshape(64, G_xe, GRP).transpose(1, 0, 2))     # [G_xe, 64, GRP]
        smat = np.zeros((128, S_s), S_NP)
        smat[s_rows[c], s_cols[c]] = 1.0
        smat = np.ascontiguousarray(
            smat.reshape(128, G_s, GRP).transpose(1, 0, 2))   # [G_s, 128, GRP]
        xtown = np.zeros((64, SHARD_PAD), ml_dtypes.bfloat16)
        xtown[:, :SHARD] = xm[c * SHARD:(c + 1) * SHARD].T
        in_maps.append({
            "xe": xeg, "smat": smat, "xtown": xtown,
            "attT": attT, "bas": bas,
        })

    import os
    trace = os.environ.get("K_TRACE", "0") == "1"
    if trace:
        _install_ntff_shim()
    res = bass_utils.run_bass_kernel_spmd(
        nc, in_maps, core_ids=list(range(N_CORES)), trace=trace)
    kernel.last_res = res
    if trace and res.exec_time_ns is not None:
        print(f"HW exec time: {res.exec_time_ns} ns", flush=True)
        kernel.last_exec_time_ns = res.exec_time_ns

    out = np.zeros((N_NODES, D_OUT), np.float32)
    for c in range(N_CORES):
        o = np.asarray(res.results[c]["out"], np.float32)   # [128, N_BLK, 64]
        out[c * SHARD:(c + 1) * SHARD] = (
            o.transpose(1, 0, 2).reshape(SHARD_PAD, D_OUT)[:SHARD])
    return out

